# revision 3
# baseline (speedup 1.0000x reference)
"""Causal multi-head attention on 8 Trainium2 NeuronCores.

Sharding: data-parallel over batch (B=2) x tensor-parallel over heads
(16 heads -> 4 groups of 4). Core c handles batch c//4, heads
[4*(c%4), 4*(c%4)+4). Each core computes its head-slice QKV projections,
causal softmax attention, and a partial output projection (row-sharded
Wo). The host sums the 4 partials per batch and adds the biases that
commute with the reduction (bo + Wo @ bv).

v2 structural changes over the first working kernel:
  - DMA preamble: x tiles go x0-first on the sync ring; weights are
    split so the first projection matmul depends only on the 64KB dt=0
    slice of Wq (not the full 1MB image). Biases ride the scalar ring
    after the first two weight slices.
  - Projections run in four ft-sub-passes (Wq-ft0, Wq-ft1, Wk-ft0,
    Wk-ft1), dt-outer within each, 4 PSUM banks per sub-pass, so PE
    work starts as soon as the first weight slice + x0 land and
    evictions of one sub-pass overlap the next sub-pass's matmuls.
  - qT/kT/aT live as per-qc chunk tiles so attention's first scores
    only wait on the qc0 evictions of the last sub-pass.
  - exp is always a single full-width ACT instruction per score pair
    (stale PSUM cols are bounded old scores; p@v skips junk cols), and
    the softmax normalize chain runs reciprocal on the [1,QC] sum row
    straight out of PSUM before broadcasting, cutting ~30us of DVE.
  - output projection is qb-granular, deferred one q-chunk and
    interleaved between head-pair groups to fill normalize-chain gaps;
    output DMA is bf16 (host upcasts) halving output traffic.
"""

import os

os.environ.setdefault("MYCRO_LOCAL_CACHE", "1")

import ml_dtypes
import numpy as np

import concourse.bass as bass
import concourse.tile as tile
from concourse import bacc, mybir
from concourse.bass import ds, ts
from concourse.bass_utils import run_bass_kernel_spmd

AF = mybir.ActivationFunctionType

B = 2
S = 2048
D = 1024
N_HEADS = 16
DH = 64
N_CORES = 8

HG = 4            # heads per core
FH = HG * DH      # 256 features per core
P = 128
NFT = FH // P     # 2 f-tiles per core
NDT = D // P      # 8 d_model tiles
QC = 512          # q chunk (moving free dim)
NQC = S // QC     # 4
KT = 128          # k tile (partition dim of sT)
NKT = S // KT     # 16
NEH = D // QC     # 2 output-projection column halves
QBP = QC // P     # 4 q-blocks of 128 per q-chunk

F32 = mybir.dt.float32
F32R = mybir.dt.float32r
BF16 = mybir.dt.bfloat16

MMDT = BF16


def to_mmdt(a):
    """Host-side cast to the matmul operand dtype."""
    a = np.ascontiguousarray(np.asarray(a, np.float32))
    if MMDT == BF16:
        return np.ascontiguousarray(a.astype(ml_dtypes.bfloat16))
    return a


def build_program():
    nc = bacc.Bacc(None, target_bir_lowering=False)

    # DRAM images are the exact SBUF layouts (partition-major) so each
    # partition's data is one contiguous >=4KB run.
    x_d = nc.dram_tensor("x_img", [P, NDT * S], MMDT, kind="ExternalInput")
    wq_d = nc.dram_tensor("wq_img", [P, NDT * FH], MMDT, kind="ExternalInput")
    wk_d = nc.dram_tensor("wk_img", [P, NDT * FH], MMDT, kind="ExternalInput")
    wv_d = nc.dram_tensor("wv_img", [P, NDT * FH], MMDT, kind="ExternalInput")
    wo_d = nc.dram_tensor("wo_img", [P, NFT * D], MMDT, kind="ExternalInput")
    bq_d = nc.dram_tensor("bq2", [P, NFT], F32, kind="ExternalInput")
    bk_d = nc.dram_tensor("bk2", [P, NFT], F32, kind="ExternalInput")
    out_d = nc.dram_tensor("out", [S, D], MMDT, kind="ExternalOutput")

    with tile.TileContext(nc) as tc:
        with tc.tile_pool(name="persist", bufs=1) as persist:
            # per-qc chunk tiles for fine-grained dependencies
            qT_ch = [persist.tile([P, NFT, QC], MMDT, name=f"qTc{q}") for q in range(NQC)]
            kT_ch = [persist.tile([P, NFT, QC], MMDT, name=f"kTc{q}") for q in range(NQC)]
            aT_ch = [persist.tile([P, NFT, QC], MMDT, name=f"aTc{q}") for q in range(NQC)]
            v_sb = persist.tile([P, NKT, HG, DH + 1], MMDT)
            wo_sb = persist.tile([P, NFT, D], MMDT)
            bq_sb = persist.tile([P, NFT], F32)
            bk_sb = persist.tile([P, NFT], F32)

            # ---- DMA issue order is the preamble-latency knob ----
            # sync ring: x tiles, x0 first (first projection input).
            x_dt = []
            for dt in range(NDT):
                xt = persist.tile([P, S], MMDT, name=f"x{dt}")
                nc.sync.dma_start(xt[:], x_d[:, ts(dt, S)])
                x_dt.append(xt)
            # scalar ring: dt0 slices of Wq/Wk first (64KB each) so the
            # first matmul starts ~10us earlier than with whole-image DMAs.
            wq0 = persist.tile([P, FH], MMDT, name="wq0")
            wk0 = persist.tile([P, FH], MMDT, name="wk0")
            wqr = persist.tile([P, NDT - 1, FH], MMDT, name="wqr")
            wkr = persist.tile([P, NDT - 1, FH], MMDT, name="wkr")
            wv_sb = persist.tile([P, NDT, FH], MMDT, name="wv")
            nc.scalar.dma_start(wq0[:], wq_d[:, 0:FH])
            nc.scalar.dma_start(wk0[:], wk_d[:, 0:FH])
            nc.scalar.dma_start(
                wqr[:], wq_d[:, FH:].rearrange("p (dt f) -> p dt f", f=FH)
            )
            nc.scalar.dma_start(
                wkr[:], wk_d[:, FH:].rearrange("p (dt f) -> p dt f", f=FH)
            )
            nc.scalar.dma_start(bq_sb[:], bq_d[:])
            nc.scalar.dma_start(bk_sb[:], bk_d[:])
            nc.scalar.dma_start(wv_sb[:], wv_d[:].rearrange("p (dt f) -> p dt f", f=FH))
            nc.scalar.dma_start(wo_sb[:], wo_d[:].rearrange("p (ft e) -> p ft e", e=D))

            nc.vector.memset(v_sb[:, :, :, DH : DH + 1], 1.0)

            # triangle mask tile (keep k<=q) for the causal diagonal
            tri = persist.tile([P, KT], MMDT)
            nc.vector.memset(tri[:], 1.0)
            nc.gpsimd.affine_select(
                out=tri[:],
                in_=tri[:],
                compare_op=mybir.AluOpType.is_ge,
                fill=0.0,
                base=0,
                channel_multiplier=-1,
                pattern=[[1, KT]],
            )
            # ACT exp-table preload, after the scalar ring's DMA issues
            warm = persist.tile([P, 16], F32)
            nc.vector.memset(warm[:], 0.0)
            nc.scalar.activation(warm[:], warm[:], AF.Exp)

            # ---------------- QK projections ----------------
            # Four sub-passes of 4 PSUM banks each; evictions of one
            # sub-pass overlap the next sub-pass's matmuls.
            with tc.tile_pool(name="psum_p", bufs=1, space=bass.MemorySpace.PSUM) as pp:
                for w0, wr, b_sb, dst in (
                    (wq0, wqr, bq_sb, qT_ch),
                    (wk0, wkr, bk_sb, kT_ch),
                ):
                    for ft in range(NFT):
                        pj = [
                            pp.tile([P, QC], F32, tag="pj", bufs=8, name=f"pj{id(w0)}_{ft}_{qc}")
                            for qc in range(NQC)
                        ]
                        for dt in range(NDT):
                            wsrc = w0[:, ts(ft, P)] if dt == 0 else wr[:, dt - 1, ts(ft, P)]
                            for qc in range(NQC):
                                nc.tensor.matmul(
                                    pj[qc][:],
                                    wsrc,
                                    x_dt[dt][:, ts(qc, QC)],
                                    start=(dt == 0),
                                    stop=(dt == NDT - 1),
                                )
                        for qc in range(NQC):
                            nc.scalar.activation(
                                dst[qc][:, ft, :],
                                pj[qc][:],
                                AF.Identity,
                                bias=b_sb[:, ft : ft + 1],
                            )

            # ---------------- attention + output projection ----------------
            with (
                tc.tile_pool(name="attn_sb", bufs=4) as ap_pool,
                tc.tile_pool(name="psum_s", bufs=2, space=bass.MemorySpace.PSUM) as ps_pool,
                tc.tile_pool(name="psum_a", bufs=2, space=bass.MemorySpace.PSUM) as pa_pool,
                tc.tile_pool(name="norm", bufs=4) as norm_pool,
                tc.tile_pool(name="psum_o", bufs=2, space=bass.MemorySpace.PSUM) as po_pool,
                tc.tile_pool(name="out_sb", bufs=3) as ot_pool,
            ):

                def out_proj_qb(qb):
                    # output projection for one finished 128-row q-block
                    qcf = qb // QBP
                    qo = (qb % QBP) * P
                    pos = [
                        po_pool.tile([P, QC], F32, tag="po", name=f"po{qb}_{eh}")
                        for eh in range(NEH)
                    ]
                    for ft in range(NFT):
                        for eh in range(NEH):
                            nc.tensor.matmul(
                                pos[eh][:],
                                aT_ch[qcf][:, ft, ds(qo, P)],
                                wo_sb[:, ft, ts(eh, QC)],
                                start=(ft == 0),
                                stop=(ft == NFT - 1),
                            )
                    ot = ot_pool.tile([P, D], MMDT, tag="ot", name=f"ot{qb}")
                    for eh in range(NEH):
                        nc.vector.tensor_copy(ot[:, ts(eh, QC)], pos[eh][:])
                    nc.sync.dma_start(out_d[ts(qb, P), :], ot[:])

                for qi, qc in enumerate(range(NQC)):
                    nkt = (qc + 1) * (QC // KT)
                    # just-in-time v projection for this q-range's new k-tiles
                    for kt in range(qc * (QC // KT), nkt):
                        pv = po_pool.tile([P, FH], F32, tag="po", name=f"pv{kt}")
                        for dt in range(NDT):
                            nc.tensor.matmul(
                                pv[:],
                                x_dt[dt][:, ts(kt, KT)],
                                wv_sb[:, dt, :],
                                start=(dt == 0),
                                stop=(dt == NDT - 1),
                            )
                        nc.vector.tensor_copy(
                            v_sb[:, kt, :, 0:DH],
                            pv[:].rearrange("p (h d) -> p h d", h=HG),
                        )
                    for hp in range(NFT):
                        heads = (2 * hp, 2 * hp + 1)
                        psas = {
                            h: pa_pool.tile([DH + 1, QC], F32, tag="psa", name=f"psa{h}_{qc}")
                            for h in heads
                        }
                        pending = []

                        def flush_one():
                            h_, pt_, cc_ = pending.pop(0)
                            for u_, (kt_, t_, c0_) in enumerate(cc_):
                                nc.tensor.matmul(
                                    psas[h_][:, ds(c0_, QC - c0_)],
                                    v_sb[:, kt_, h_, :],
                                    pt_[:, ds(u_ * QC + c0_, QC - c0_)],
                                    start=(kt_ == 0),
                                    stop=(kt_ == nkt - 1),
                                )

                        for ktp in range(0, nkt, 2):
                            cc = []
                            for u in (0, 1):
                                kt = ktp + u
                                t = kt - qc * (QC // KT)
                                c0 = KT * t if t > 0 else 0
                                cc.append((kt, t, c0))
                            tiles = {
                                h: (
                                    ps_pool.tile(
                                        [P, 2 * QC], F32, tag="pss", name=f"pss{h}_{qc}_{ktp}"
                                    ),
                                    ap_pool.tile(
                                        [P, 2 * QC], MMDT, tag="pt", name=f"pt{h}_{qc}_{ktp}"
                                    ),
                                )
                                for h in heads
                            }
                            # scores: alternate heads per matmul so weight
                            # loads land in the other head's row group
                            for u, (kt, t, c0) in enumerate(cc):
                                for h in heads:
                                    pb = DH * (h % 2)
                                    pss, pt = tiles[h]
                                    nc.tensor.matmul(
                                        pss[:, ds(u * QC + c0, QC - c0)],
                                        kT_ch[kt // QBP][pb : pb + DH, hp, ts(kt % QBP, KT)],
                                        qT_ch[qc][pb : pb + DH, hp, ds(c0, QC - c0)],
                                        start=True,
                                        stop=True,
                                    )
                            for h in heads:
                                pss, pt = tiles[h]
                                # one full-width exp per pair: junk cols in
                                # [a, ...] hold bounded stale scores; p@v
                                # slices them away, the diagonal triangle is
                                # zeroed below.
                                a = cc[0][2]
                                nc.scalar.activation(
                                    pt[:, ds(a, 2 * QC - a)],
                                    pss[:, ds(a, 2 * QC - a)],
                                    AF.Exp,
                                )
                                for u, (kt, t, c0) in enumerate(cc):
                                    if t >= 0:
                                        # zero the still-masked triangle
                                        reg = pt[:, ds(u * QC + c0, KT)]
                                        nc.vector.tensor_mul(reg, reg, tri[:])
                                pending.append((h, pt, cc))
                                while len(pending) > 2:
                                    flush_one()
                        while pending:
                            flush_one()

                        # normalize: reciprocal of the denominator row
                        # straight out of PSUM, broadcast, then scale the
                        # raw attention rows into aT.
                        nt = {}
                        for h in heads:
                            nt[h] = (
                                norm_pool.tile([1, QC], F32, tag="se", bufs=4, name=f"se{h}_{qc}"),
                                norm_pool.tile([1, QC], F32, tag="rc1", bufs=4, name=f"rc1{h}_{qc}"),
                                norm_pool.tile([DH, QC], F32, tag="rcb", bufs=4, name=f"rcb{h}_{qc}"),
                            )
                        for h in heads:
                            # copy the denominator row out of PSUM (partition
                            # 64 -> 0), then reciprocal on [1,QC], broadcast
                            # the reciprocal, and scale the raw rows.
                            nc.vector.tensor_copy(nt[h][0][:], psas[h][DH : DH + 1, :])
                        for h in heads:
                            nc.vector.reciprocal_approx_fast(nt[h][1][:], nt[h][0][:])
                        for h in heads:
                            nc.gpsimd.partition_broadcast(nt[h][2][:], nt[h][1][:])
                        for h in heads:
                            pb = DH * (h % 2)
                            nc.vector.tensor_mul(
                                aT_ch[qc][pb : pb + DH, hp, :],
                                psas[h][0:DH, :],
                                nt[h][2][:],
                            )

                        # interleave finished q-blocks' output projections
                        # between head-pair groups (1-qc deferral)
                        if qi >= 1:
                            base = (qi - 1) * QBP
                            if hp == 0:
                                out_proj_qb(base + 0)
                                out_proj_qb(base + 1)
                            else:
                                out_proj_qb(base + 2)
                                out_proj_qb(base + 3)

                for qb in range((NQC - 1) * QBP, NQC * QBP):
                    out_proj_qb(qb)

    nc.finalize()
    return nc


_NC_CACHE = {}


def get_program():
    if "nc" not in _NC_CACHE:
        _NC_CACHE["nc"] = build_program()
    return _NC_CACHE["nc"]


def _img(a, nt):
    """[nt*P, F] -> partition-major SBUF image [P, nt*F]."""
    ntp, f = a.shape
    assert ntp == nt * P
    return np.ascontiguousarray(
        a.reshape(nt, P, f).transpose(1, 0, 2).reshape(P, nt * f)
    )


def shard_inputs(x, mask, Wq, bq, Wk, bk, Wv, bv, Wo, bo):
    """Build the per-core input maps (host-side layout prep only)."""
    del mask  # causality is structural in the kernel
    in_maps = []
    for c in range(N_CORES):
        b = c // 4
        g = c % 4
        fsl = slice(FH * g, FH * (g + 1))
        in_maps.append(
            {
                "x_img": _img(to_mmdt(x[b].T), NDT),
                "wq_img": _img(to_mmdt(Wq[fsl, :].T / 8.0), NDT),
                "wk_img": _img(to_mmdt(Wk[fsl, :].T), NDT),
                "wv_img": _img(to_mmdt(Wv[fsl, :].T), NDT),
                "wo_img": _img(to_mmdt(Wo[:, fsl].T), NFT),
                "bq2": np.ascontiguousarray(
                    (bq[fsl] / 8.0).reshape(NFT, P).T.astype(np.float32)
                ),
                "bk2": np.ascontiguousarray(
                    bk[fsl].reshape(NFT, P).T.astype(np.float32)
                ),
            }
        )
    return in_maps


def gather_outputs(results, bias_term):
    """Sum the head-group partials per batch and add the folded biases."""
    out = np.zeros((B, S, D), dtype=np.float32)
    for b in range(B):
        acc = results[4 * b]["out"].astype(np.float32)
        for g in range(1, 4):
            acc = acc + results[4 * b + g]["out"].astype(np.float32)
        out[b] = acc + bias_term
    return out


def kernel(x, mask, Wq, bq, Wk, bk, Wv, bv, Wo, bo, **run_kwargs):
    x = np.asarray(x)
    mask = np.asarray(mask)
    Wq, bq = np.asarray(Wq), np.asarray(bq)
    Wk, bk = np.asarray(Wk), np.asarray(bk)
    Wv, bv = np.asarray(Wv), np.asarray(bv)
    Wo, bo = np.asarray(Wo), np.asarray(bo)

    nc = get_program()
    in_maps = shard_inputs(x, mask, Wq, bq, Wk, bk, Wv, bv, Wo, bo)
    res = run_bass_kernel_spmd(nc, in_maps, core_ids=list(range(N_CORES)), **run_kwargs)
    # bias term that commutes with the cross-core reduction:
    # out += bo + Wo @ bv  (bv's effect on attention output is +bv per
    # feature after softmax normalization)
    bias_term = (bo.astype(np.float32) + Wo.astype(np.float32) @ bv.astype(np.float32))
    out = gather_outputs(res.results, bias_term)
    kernel.last_results = res
    return out
